# revision 22
# baseline (speedup 1.0000x reference)
"""ComplexAttentionV3 Trainium2 kernel (v3).

Sharding: 8 cores = data-parallel over batch (2) x tensor-parallel over
heads (16 -> 4 per core). Each core computes q/k/v for its 4 heads
(column-sharded projections), local attention, and a row-sharded
o-projection producing a partial [T, D] output; the host sums the 4
partials per batch.

v3 notes vs v2 (559us baseline):
- attention software-pipelined: scores for pair p+1 are emitted before
  the AV matmuls of pair p, so the PE never idles waiting for the ACT
  exp (the 822ns/2jc stall + the resulting p-state down-clock were the
  dominant cost in v2's 272us attention phase);
- softmax denominator fused into the AV matmul as a 65th lhsT column of
  ones (v_real | 1), removing the separate dn matmuls' PSUM pool and
  freeing banks for double-buffered accumulators (av pools bufs=2), so
  the per-(h,iw) normalize chain (DVE recip + gpsimd broadcast + muls)
  runs off the PE critical path;
- attention works on 512-query windows: scores pair tile [128,1024]
  (2 banks, bufs=2) + avr [65,512] + avi [64,512] (1 bank, bufs=2 each)
  = 8 PSUM banks exactly;
- o-proj weights DMA'd at attention start (v2 left them to the o-proj
  phase and stalled 10us); outputs alternate between the sync and
  scalar HWDGE queues (v2 pushed all 16MB through one queue);
- x DMA'd in 512-column slabs interleaved r/i to match first-use order.
"""

import numpy as np
import ml_dtypes

import concourse.bacc as bacc
import concourse.tile as tile
from concourse import mybir
from concourse.bass import ts
from concourse.bass_utils import run_bass_kernel_spmd

B, T, D, H = 2, 2048, 1024, 16
HD = 64
NCORE = 8
TP = 4               # head-parallel degree (per batch)
HC = H // TP         # heads per core = 4
C = HC * HD          # local channels = 256
DC = D // 128        # contraction chunks = 8
TQ = T // 128        # 128-row t-chunks = 16
T5 = T // 512        # 512-col t-chunks = 4
TW = T // 1024       # 1024-col t-chunks = 2
NP = TQ // 2         # key-chunk pairs = 8

F32 = mybir.dt.float32
BF16 = mybir.dt.bfloat16
EXP = mybir.ActivationFunctionType.Exp

LAST_RESULTS = None
_COMPILED = None


def _build():
    nc = bacc.Bacc("TRN2", target_bir_lowering=False, debug=False,
                   num_devices=NCORE)

    def din(name, shape, dt=BF16):
        return nc.dram_tensor(name, shape, dt, kind="ExternalInput").ap()

    xr_d = din("xrT", [128, T5, DC, 512])
    xi_d = din("xiT", [128, T5, DC, 512])
    wq = {k: din(f"wq_{k}", [128, DC, C]) for k in ("r", "i", "n")}
    wk = {k: din(f"wk_{k}", [128, DC, C]) for k in ("r", "i", "n")}
    wv = {k: din(f"wv_{k}", [128, DC, 2 * C]) for k in ("a", "b")}
    ow = {k: din(f"ow_{k}", [128, 2, D]) for k in ("r", "i", "n")}
    cos_d = din("cos2", [128, T], F32)
    sin_d = din("sin2", [128, T], F32)
    outr_d = nc.dram_tensor("out_r", [T, D], F32, kind="ExternalOutput").ap()
    outi_d = nc.dram_tensor("out_i", [T, D], F32, kind="ExternalOutput").ap()

    with tile.TileContext(nc) as tc:
        with tc.tile_pool(name="persist", bufs=1) as persist:
            qkcat = persist.tile([128, 2 * HC, T], BF16, name="qkcat")
            # v_real and v_imag each padded to 65 columns per (key-chunk,
            # head): column 64 is ones. For v_real it computes the softmax
            # denominator into avr partition 64; for v_imag it only pads M
            # to 65 so the matmul stays in the PE's 128-column tile mode
            # (M=64 selects the 64-column mode and every mode switch costs
            # ~95ns). Flat index is tq * HC + h.
            vcr = persist.tile([128, TQ * HC, 65], BF16, name="vcr")
            vci = persist.tile([128, TQ * HC, 65], BF16, name="vci")
            # per-512-query-window u tiles (separate tiles so the o-proj's
            # reads don't pick up a false whole-tile dependency on the last
            # attention window)
            urts = [persist.tile([128, 2, 512], BF16, name=f"urt{w}")
                    for w in range(T5)]
            uits = [persist.tile([128, 2, 512], BF16, name=f"uit{w}")
                    for w in range(T5)]
            nc.vector.memset(vcr[:, :, 64:65], 1.0)
            nc.vector.memset(vci[:, :, 64:65], 1.0)

            # ---------------- projection phase ----------------
            with tc.tile_pool(name="xw", bufs=1) as xw, \
                 tc.tile_pool(name="rt", bufs=1) as rt, \
                 tc.tile_pool(name="pp", bufs=2, space="PSUM") as pp:
                # scalar HWDGE queue: weights + rope tables in first-use
                # order; sync HWDGE queue: x in 512-col slabs, r/i
                # interleaved (q-proj consumes slab s of xr then xi).
                wqs = {k: xw.tile([128, DC, C], BF16, name=f"wq{k}")
                       for k in ("r", "i", "n")}
                wks = {k: xw.tile([128, DC, C], BF16, name=f"wk{k}")
                       for k in ("r", "i", "n")}
                wvs = {k: xw.tile([128, DC, 2 * C], BF16, name=f"wv{k}")
                       for k in ("a", "b")}
                cos = xw.tile([128, T], F32, name="cos")
                sin = xw.tile([128, T], F32, name="sin")
                for k in ("r", "i", "n"):
                    nc.scalar.dma_start(wqs[k][:], wq[k][:])
                nc.scalar.dma_start(cos[:], cos_d[:])
                nc.scalar.dma_start(sin[:], sin_d[:])
                for k in ("r", "i", "n"):
                    nc.scalar.dma_start(wks[k][:], wk[k][:])
                for k in ("a", "b"):
                    nc.scalar.dma_start(wvs[k][:], wv[k][:])
                xr = xw.tile([128, T5, DC, 512], BF16, name="xr")
                xi = xw.tile([128, T5, DC, 512], BF16, name="xi")
                # xr on the sync HWDGE queue, xi on the gpsimd SWDGE queue:
                # two queues halve the time to the first full slab pair.
                for sl in range(T5):
                    nc.sync.dma_start(xr[:, sl], xr_d[:, sl])
                    nc.gpsimd.dma_start(xi[:, sl], xi_d[:, sl])

                # q/k projections (transposed [c, t]) + RoPE into qkcat
                for wsrc, hbase in ((wqs, 0), (wks, HC)):
                    for cc in range(2):
                        h0, h1 = hbase + 2 * cc, hbase + 2 * cc + 1
                        for tw in range(TW):
                            pqr = pp.tile([128, 1024], F32, name="ppa")
                            pqi = pp.tile([128, 1024], F32, name="ppb")
                            for half in range(2):
                                sl = 2 * tw + half
                                psl = ts(half, 512)
                                # both xr-consuming chains before the
                                # xi-consuming ones: 16 matmuls of work
                                # while the xi slab DMA is still in flight
                                for dc in range(DC):
                                    nc.tensor.matmul(
                                        pqr[:, psl],
                                        lhsT=wsrc["r"][:, dc, ts(cc, 128)],
                                        rhs=xr[:, sl, dc, :],
                                        start=(dc == 0), stop=False)
                                for dc in range(DC):
                                    nc.tensor.matmul(
                                        pqi[:, psl],
                                        lhsT=wsrc["i"][:, dc, ts(cc, 128)],
                                        rhs=xr[:, sl, dc, :],
                                        start=(dc == 0), stop=False)
                                for dc in range(DC):
                                    nc.tensor.matmul(
                                        pqr[:, psl],
                                        lhsT=wsrc["n"][:, dc, ts(cc, 128)],
                                        rhs=xi[:, sl, dc, :],
                                        start=False, stop=(dc == DC - 1))
                                for dc in range(DC):
                                    nc.tensor.matmul(
                                        pqi[:, psl],
                                        lhsT=wsrc["r"][:, dc, ts(cc, 128)],
                                        rhs=xi[:, sl, dc, :],
                                        start=False, stop=(dc == DC - 1))
                            tsl = ts(tw, 1024)
                            t1 = rt.tile([128, 1024], F32, name="t1")
                            t2 = rt.tile([128, 1024], F32, name="t2")
                            t3 = rt.tile([128, 1024], F32, name="t3")
                            t4 = rt.tile([128, 1024], F32, name="t4")
                            nc.vector.tensor_mul(t1[:], pqr[:], cos[:, tsl])
                            nc.vector.tensor_mul(t2[:], pqi[:], sin[:, tsl])
                            nc.vector.tensor_mul(t3[:], pqr[:], sin[:, tsl])
                            nc.vector.tensor_mul(t4[:], pqi[:], cos[:, tsl])
                            nc.vector.tensor_sub(qkcat[0:64, h0, tsl],
                                                 t1[0:64, :], t2[0:64, :])
                            nc.vector.tensor_sub(qkcat[0:64, h1, tsl],
                                                 t1[64:128, :], t2[64:128, :])
                            nc.vector.tensor_add(qkcat[64:128, h0, tsl],
                                                 t3[0:64, :], t4[0:64, :])
                            nc.vector.tensor_add(qkcat[64:128, h1, tsl],
                                                 t3[64:128, :], t4[64:128, :])

                # v projection: natural [t, c], rhs packed [wvr | wvi]
                for tq in range(TQ):
                    pv = pp.tile([128, 1024], F32, name="ppa")
                    pvs = pv[:, 0:512]
                    w, off = tq // 4, (tq % 4) * 128
                    for dc in range(DC):
                        nc.tensor.matmul(pvs,
                                         lhsT=xr[:, w, dc, off:off + 128],
                                         rhs=wvs["a"][:, dc, :],
                                         start=(dc == 0), stop=False)
                    for dc in range(DC):
                        nc.tensor.matmul(pvs,
                                         lhsT=xi[:, w, dc, off:off + 128],
                                         rhs=wvs["b"][:, dc, :],
                                         start=False, stop=(dc == DC - 1))
                    nc.scalar.copy(
                        vcr[:, tq * HC:(tq + 1) * HC, 0:64],
                        pv[:, 0:C].rearrange("p (h d) -> p h d", h=HC))
                    nc.scalar.copy(
                        vci[:, tq * HC:(tq + 1) * HC, 0:64],
                        pv[:, C:2 * C].rearrange("p (h d) -> p h d", h=HC))

            # ---------------- attention phase ----------------
            # per (head, 512-query window): 8 key-chunk pairs; scores for
            # pair p+1 are emitted before the AV matmuls of pair p so the
            # exp latency is hidden behind ~1.3us of PE work.
            with tc.tile_pool(name="ox", bufs=1) as ox:
                # prefetch o-proj weights now: the scalar queue is idle and
                # SBUF has room once the projection pools wind down.
                ows = {k: ox.tile([128, 2, D], BF16, name=f"ow{k}")
                       for k in ("r", "i", "n")}
                for k in ("r", "i", "n"):
                    nc.scalar.dma_start(ows[k][:], ow[k][:])

                with tc.tile_pool(name="att", bufs=3) as att, \
                     tc.tile_pool(name="attsm", bufs=2) as attsm, \
                     tc.tile_pool(name="ost", bufs=3) as ost, \
                     tc.tile_pool(name="sp", bufs=2, space="PSUM") as sp, \
                     tc.tile_pool(name="avr", bufs=2, space="PSUM") as avrp, \
                     tc.tile_pool(name="avi", bufs=2, space="PSUM") as avip:
                    # one flat software pipeline over every (window, head,
                    # key-chunk-pair): the scores+exp of pair g+1 are always
                    # in flight while the AV matmuls of pair g run, including
                    # across head/window boundaries, so the PE never waits on
                    # the ACT exp.
                    pairs = [(iw, h, p) for iw in range(T5)
                             for h in range(HC) for p in range(NP)]
                    etiles, avst = {}, {}

                    def emit_scores(g):
                        iw, h, p = pairs[g]
                        s = sp.tile([128, 1024], F32, name="s")
                        for j in range(2):
                            nc.tensor.matmul(
                                s[:, ts(j, 512)],
                                lhsT=qkcat[:, HC + h, ts(2 * p + j, 128)],
                                rhs=qkcat[:, h, ts(iw, 512)],
                                start=True, stop=True)
                        es = att.tile([128, 1024], BF16, name="es")
                        nc.scalar.activation(es[:], s[:], EXP, scale=0.125)
                        etiles[g] = es

                    def emit_accum(g):
                        iw, h, p = pairs[g]
                        if p == 0:
                            avst[(iw, h)] = (
                                avrp.tile([65, 512], F32, name="avr"),
                                avip.tile([65, 512], F32, name="avi"))
                        avr, avi = avst[(iw, h)]
                        es = etiles.pop(g)
                        for j in range(2):
                            nc.tensor.matmul(
                                avr[:], lhsT=vcr[:, (2 * p + j) * HC + h, :],
                                rhs=es[:, ts(j, 512)],
                                start=(p == 0 and j == 0),
                                stop=(p == NP - 1 and j == 1))
                        for j in range(2):
                            nc.tensor.matmul(
                                avi[:], lhsT=vci[:, (2 * p + j) * HC + h, :],
                                rhs=es[:, ts(j, 512)],
                                start=(p == 0 and j == 0),
                                stop=(p == NP - 1 and j == 1))
                        if p == NP - 1:
                            ucc, up0 = h // 2, (h % 2) * 64
                            dnr = attsm.tile([1, 512], F32, name="dnr")
                            nc.scalar.copy(dnr[:], avr[64:65, :])
                            rec = attsm.tile([1, 512], F32, name="rec")
                            nc.vector.reciprocal_approx_fast(rec[:], dnr[:])
                            bc = attsm.tile([128, 512], F32, name="bc")
                            nc.gpsimd.partition_broadcast(bc[:], rec[:])
                            nc.vector.tensor_mul(
                                urts[iw][up0:up0 + 64, ucc, :],
                                avr[0:64, :], bc[0:64, :])
                            nc.vector.tensor_mul(
                                uits[iw][up0:up0 + 64, ucc, :],
                                avi[0:64, :], bc[64:128, :])

                    emit_scores(0)
                    for g in range(len(pairs)):
                        if g + 1 < len(pairs):
                            emit_scores(g + 1)
                        emit_accum(g)

                    # ---------------- output projection ----------------
                    # PSUM for por/poi comes from the sp pool (same shape,
                    # same buffers) so no new PSUM pool has to wait for the
                    # attention pools to drain.
                    for tq in range(TQ):
                        tslq = ts(tq, 128)
                        w, off = tq // 4, (tq % 4) * 128
                        ur, ui = urts[w], uits[w]
                        usl = slice(off, off + 128)
                        por = sp.tile([128, 1024], F32, name="s")
                        poi = sp.tile([128, 1024], F32, name="s")
                        for oc in range(2):
                            osl = ts(oc, 512)
                            nc.tensor.matmul(por[:, osl], lhsT=ur[:, 0, usl],
                                             rhs=ows["r"][:, 0, osl],
                                             start=True, stop=False)
                            nc.tensor.matmul(por[:, osl], lhsT=ur[:, 1, usl],
                                             rhs=ows["r"][:, 1, osl],
                                             start=False, stop=False)
                            nc.tensor.matmul(por[:, osl], lhsT=ui[:, 0, usl],
                                             rhs=ows["n"][:, 0, osl],
                                             start=False, stop=False)
                            nc.tensor.matmul(por[:, osl], lhsT=ui[:, 1, usl],
                                             rhs=ows["n"][:, 1, osl],
                                             start=False, stop=True)
                            nc.tensor.matmul(poi[:, osl], lhsT=ur[:, 0, usl],
                                             rhs=ows["i"][:, 0, osl],
                                             start=True, stop=False)
                            nc.tensor.matmul(poi[:, osl], lhsT=ur[:, 1, usl],
                                             rhs=ows["i"][:, 1, osl],
                                             start=False, stop=False)
                            nc.tensor.matmul(poi[:, osl], lhsT=ui[:, 0, usl],
                                             rhs=ows["r"][:, 0, osl],
                                             start=False, stop=False)
                            nc.tensor.matmul(poi[:, osl], lhsT=ui[:, 1, usl],
                                             rhs=ows["r"][:, 1, osl],
                                             start=False, stop=True)
                        st = ost.tile([128, 1024], F32, name="st")
                        nc.scalar.copy(st[:], por[:])
                        nc.sync.dma_start(outr_d[tslq, :], st[:])
                        sti = ost.tile([128, 1024], F32, name="sti")
                        nc.vector.tensor_copy(sti[:], poi[:])
                        nc.scalar.dma_start(outi_d[tslq, :], sti[:])

    nc.compile()
    return nc


def _to_bf16_kxm(arr, parts=128):
    """[K, M] fp32 -> [128, K//128, M] bf16 with K split as (chunk, part)."""
    k, m = arr.shape
    out = arr.reshape(k // parts, parts, m).transpose(1, 0, 2)
    return np.ascontiguousarray(out.astype(ml_dtypes.bfloat16))


def _to_x_slabs(arr):
    """[T, D] fp32 -> [128, T5, DC, 512] bf16: D split as (chunk, part),
    T split into 512-col slabs, slab-major so each slab is contiguous."""
    out = _to_bf16_kxm(arr.T.astype(np.float32))        # [128, DC, T]
    out = out.reshape(128, DC, T5, 512).transpose(0, 2, 1, 3)
    return np.ascontiguousarray(out)


def _rope_tables():
    inv_freq = 1.0 / (10000.0 ** (np.arange(0, HD, 2, dtype=np.float64) / HD))
    invf64 = np.concatenate([inv_freq, inv_freq])          # [64]
    ang = invf64[:, None] * np.arange(T, dtype=np.float64)[None, :]  # [64, T]
    cos2 = np.tile(np.cos(ang), (2, 1)).astype(np.float32)
    sin2 = np.tile(np.sin(ang), (2, 1)).astype(np.float32)
    return np.ascontiguousarray(cos2), np.ascontiguousarray(sin2)


def kernel(x_real, x_imag, q_wr, q_wi, k_wr, k_wi, v_wr, v_wi, o_wr, o_wi):
    global _COMPILED, LAST_RESULTS
    if _COMPILED is None:
        _COMPILED = _build()
    nc = _COMPILED

    cos2, sin2 = _rope_tables()
    xt = {}
    for b in range(B):
        xt[("r", b)] = _to_x_slabs(np.asarray(x_real[b]))
        xt[("i", b)] = _to_x_slabs(np.asarray(x_imag[b]))

    in_maps = []
    for core in range(NCORE):
        b, g = core // TP, core % TP
        cols = slice(g * C, (g + 1) * C)
        m = {"xrT": xt[("r", b)], "xiT": xt[("i", b)],
             "cos2": cos2, "sin2": sin2}
        for nm, wr_, wi_ in (("wq", q_wr, q_wi), ("wk", k_wr, k_wi)):
            m[f"{nm}_r"] = _to_bf16_kxm(np.asarray(wr_[:, cols]))
            m[f"{nm}_i"] = _to_bf16_kxm(np.asarray(wi_[:, cols]))
            m[f"{nm}_n"] = _to_bf16_kxm(-np.asarray(wi_[:, cols]))
        vr_, vi_ = np.asarray(v_wr[:, cols]), np.asarray(v_wi[:, cols])
        m["wv_a"] = _to_bf16_kxm(np.concatenate([vr_, vi_], axis=1))
        m["wv_b"] = _to_bf16_kxm(np.concatenate([-vi_, vr_], axis=1))
        m["ow_r"] = _to_bf16_kxm(np.asarray(o_wr[cols, :]))
        m["ow_i"] = _to_bf16_kxm(np.asarray(o_wi[cols, :]))
        m["ow_n"] = _to_bf16_kxm(-np.asarray(o_wi[cols, :]))
        in_maps.append(m)

    res = run_bass_kernel_spmd(nc, in_maps, core_ids=list(range(NCORE)))
    LAST_RESULTS = res

    final_r = np.zeros((B, T, D), np.float32)
    final_i = np.zeros((B, T, D), np.float32)
    for core in range(NCORE):
        b = core // TP
        final_r[b] += res.results[core]["out_r"]
        final_i[b] += res.results[core]["out_i"]
    return final_r, final_i


# revision 25
# speedup vs baseline: 1.0298x; 1.0298x over previous
"""ComplexAttentionV3 Trainium2 kernel (v3).

Sharding: 8 cores = data-parallel over batch (2) x tensor-parallel over
heads (16 -> 4 per core). Each core computes q/k/v for its 4 heads
(column-sharded projections), local attention, and a row-sharded
o-projection producing a partial [T, D] output; the host sums the 4
partials per batch.

v3 notes vs v2 (559us baseline):
- attention software-pipelined: scores for pair p+1 are emitted before
  the AV matmuls of pair p, so the PE never idles waiting for the ACT
  exp (the 822ns/2jc stall + the resulting p-state down-clock were the
  dominant cost in v2's 272us attention phase);
- softmax denominator fused into the AV matmul as a 65th lhsT column of
  ones (v_real | 1), removing the separate dn matmuls' PSUM pool and
  freeing banks for double-buffered accumulators (av pools bufs=2), so
  the per-(h,iw) normalize chain (DVE recip + gpsimd broadcast + muls)
  runs off the PE critical path;
- attention works on 512-query windows: scores pair tile [128,1024]
  (2 banks, bufs=2) + avr [65,512] + avi [64,512] (1 bank, bufs=2 each)
  = 8 PSUM banks exactly;
- o-proj weights DMA'd at attention start (v2 left them to the o-proj
  phase and stalled 10us); outputs alternate between the sync and
  scalar HWDGE queues (v2 pushed all 16MB through one queue);
- x DMA'd in 512-column slabs interleaved r/i to match first-use order.
"""

import numpy as np
import ml_dtypes

import concourse.bacc as bacc
import concourse.tile as tile
from concourse import mybir
from concourse.bass import ts
from concourse.bass_utils import run_bass_kernel_spmd

B, T, D, H = 2, 2048, 1024, 16
HD = 64
NCORE = 8
TP = 4               # head-parallel degree (per batch)
HC = H // TP         # heads per core = 4
C = HC * HD          # local channels = 256
DC = D // 128        # contraction chunks = 8
TQ = T // 128        # 128-row t-chunks = 16
T5 = T // 512        # 512-col t-chunks = 4
TW = T // 1024       # 1024-col t-chunks = 2
NP = TQ // 2         # key-chunk pairs = 8

F32 = mybir.dt.float32
BF16 = mybir.dt.bfloat16
EXP = mybir.ActivationFunctionType.Exp

LAST_RESULTS = None
_COMPILED = None


def _build():
    nc = bacc.Bacc("TRN2", target_bir_lowering=False, debug=False,
                   num_devices=NCORE)

    def din(name, shape, dt=BF16):
        return nc.dram_tensor(name, shape, dt, kind="ExternalInput").ap()

    xr_d = din("xrT", [128, T5, DC, 512])
    xi_d = din("xiT", [128, T5, DC, 512])
    wq = {k: din(f"wq_{k}", [128, DC, C]) for k in ("r", "i", "n")}
    wk = {k: din(f"wk_{k}", [128, DC, C]) for k in ("r", "i", "n")}
    wv = {k: din(f"wv_{k}", [128, DC, 2 * C]) for k in ("a", "b")}
    ow = {k: din(f"ow_{k}", [128, 2, D]) for k in ("r", "i", "n")}
    cos_d = din("cos2", [128, T], F32)
    sin_d = din("sin2", [128, T], F32)
    outr_d = nc.dram_tensor("out_r", [T, D], F32, kind="ExternalOutput").ap()
    outi_d = nc.dram_tensor("out_i", [T, D], F32, kind="ExternalOutput").ap()

    with tile.TileContext(nc) as tc:
        with tc.tile_pool(name="persist", bufs=1) as persist:
            qkcat = persist.tile([128, 2 * HC, T], BF16, name="qkcat")
            # v_real and v_imag each padded to 65 columns per (key-chunk,
            # head): column 64 is ones. For v_real it computes the softmax
            # denominator into avr partition 64; for v_imag it only pads M
            # to 65 so the matmul stays in the PE's 128-column tile mode
            # (M=64 selects the 64-column mode and every mode switch costs
            # ~95ns). Flat index is tq * HC + h.
            vcr = persist.tile([128, TQ * HC, 65], BF16, name="vcr")
            vci = persist.tile([128, TQ * HC, 65], BF16, name="vci")
            # per-512-query-window u tiles (separate tiles so the o-proj's
            # reads don't pick up a false whole-tile dependency on the last
            # attention window)
            urts = [persist.tile([128, 2, 512], BF16, name=f"urt{w}")
                    for w in range(T5)]
            uits = [persist.tile([128, 2, 512], BF16, name=f"uit{w}")
                    for w in range(T5)]
            nc.vector.memset(vcr[:, :, 64:65], 1.0)
            nc.vector.memset(vci[:, :, 64:65], 1.0)

            # ---------------- projection phase ----------------
            with tc.tile_pool(name="xw", bufs=1) as xw, \
                 tc.tile_pool(name="rt", bufs=1) as rt, \
                 tc.tile_pool(name="pp", bufs=2, space="PSUM") as pp:
                # scalar HWDGE queue: weights + rope tables in first-use
                # order; sync HWDGE queue: x in 512-col slabs, r/i
                # interleaved (q-proj consumes slab s of xr then xi).
                wqs = {k: xw.tile([128, DC, C], BF16, name=f"wq{k}")
                       for k in ("r", "i", "n")}
                wks = {k: xw.tile([128, DC, C], BF16, name=f"wk{k}")
                       for k in ("r", "i", "n")}
                wvs = {k: xw.tile([128, DC, 2 * C], BF16, name=f"wv{k}")
                       for k in ("a", "b")}
                cos = xw.tile([128, T], F32, name="cos")
                sin = xw.tile([128, T], F32, name="sin")
                xr = xw.tile([128, T5, DC, 512], BF16, name="xr")
                xi = xw.tile([128, T5, DC, 512], BF16, name="xi")
                # first-needed pieces ride the sync queue in dc-halves so
                # their semaphores fire as early as possible; everything the
                # PE touches later rides the scalar queue.
                nc.sync.dma_start(wqs["r"][:, 0:4], wq["r"][:, 0:4])
                nc.sync.dma_start(wqs["r"][:, 4:8], wq["r"][:, 4:8])
                nc.sync.dma_start(xr[:, 0, 0:4], xr_d[:, 0, 0:4])
                nc.sync.dma_start(xr[:, 0, 4:8], xr_d[:, 0, 4:8])
                nc.sync.dma_start(xi[:, 0, 0:4], xi_d[:, 0, 0:4])
                nc.sync.dma_start(xi[:, 0, 4:8], xi_d[:, 0, 4:8])
                for sl in range(1, T5):
                    nc.sync.dma_start(xr[:, sl], xr_d[:, sl])
                    nc.sync.dma_start(xi[:, sl], xi_d[:, sl])
                for k in ("i", "n"):
                    nc.scalar.dma_start(wqs[k][:], wq[k][:])
                nc.scalar.dma_start(cos[:], cos_d[:])
                nc.scalar.dma_start(sin[:], sin_d[:])
                for k in ("r", "i", "n"):
                    nc.scalar.dma_start(wks[k][:], wk[k][:])
                for k in ("a", "b"):
                    nc.scalar.dma_start(wvs[k][:], wv[k][:])

                # q/k projections (transposed [c, t]) + RoPE into qkcat
                for wsrc, hbase in ((wqs, 0), (wks, HC)):
                    for cc in range(2):
                        h0, h1 = hbase + 2 * cc, hbase + 2 * cc + 1
                        for tw in range(TW):
                            pqr = pp.tile([128, 1024], F32, name="ppa")
                            pqi = pp.tile([128, 1024], F32, name="ppb")
                            for half in range(2):
                                sl = 2 * tw + half
                                psl = ts(half, 512)
                                # both xr-consuming chains before the
                                # xi-consuming ones: 16 matmuls of work
                                # while the xi slab DMA is still in flight
                                for dc in range(DC):
                                    nc.tensor.matmul(
                                        pqr[:, psl],
                                        lhsT=wsrc["r"][:, dc, ts(cc, 128)],
                                        rhs=xr[:, sl, dc, :],
                                        start=(dc == 0), stop=False)
                                for dc in range(DC):
                                    nc.tensor.matmul(
                                        pqi[:, psl],
                                        lhsT=wsrc["i"][:, dc, ts(cc, 128)],
                                        rhs=xr[:, sl, dc, :],
                                        start=(dc == 0), stop=False)
                                for dc in range(DC):
                                    nc.tensor.matmul(
                                        pqr[:, psl],
                                        lhsT=wsrc["n"][:, dc, ts(cc, 128)],
                                        rhs=xi[:, sl, dc, :],
                                        start=False, stop=(dc == DC - 1))
                                for dc in range(DC):
                                    nc.tensor.matmul(
                                        pqi[:, psl],
                                        lhsT=wsrc["r"][:, dc, ts(cc, 128)],
                                        rhs=xi[:, sl, dc, :],
                                        start=False, stop=(dc == DC - 1))
                            tsl = ts(tw, 1024)
                            t1 = rt.tile([128, 1024], F32, name="t1")
                            t2 = rt.tile([128, 1024], F32, name="t2")
                            t3 = rt.tile([128, 1024], F32, name="t3")
                            t4 = rt.tile([128, 1024], F32, name="t4")
                            nc.vector.tensor_mul(t1[:], pqr[:], cos[:, tsl])
                            nc.vector.tensor_mul(t2[:], pqi[:], sin[:, tsl])
                            nc.vector.tensor_mul(t3[:], pqr[:], sin[:, tsl])
                            nc.vector.tensor_mul(t4[:], pqi[:], cos[:, tsl])
                            nc.vector.tensor_sub(qkcat[0:64, h0, tsl],
                                                 t1[0:64, :], t2[0:64, :])
                            nc.vector.tensor_sub(qkcat[0:64, h1, tsl],
                                                 t1[64:128, :], t2[64:128, :])
                            nc.vector.tensor_add(qkcat[64:128, h0, tsl],
                                                 t3[0:64, :], t4[0:64, :])
                            nc.vector.tensor_add(qkcat[64:128, h1, tsl],
                                                 t3[64:128, :], t4[64:128, :])

                # v projection: natural [t, c], rhs packed [wvr | wvi]
                for tq in range(TQ):
                    pv = pp.tile([128, 1024], F32, name="ppa")
                    pvs = pv[:, 0:512]
                    w, off = tq // 4, (tq % 4) * 128
                    for dc in range(DC):
                        nc.tensor.matmul(pvs,
                                         lhsT=xr[:, w, dc, off:off + 128],
                                         rhs=wvs["a"][:, dc, :],
                                         start=(dc == 0), stop=False)
                    for dc in range(DC):
                        nc.tensor.matmul(pvs,
                                         lhsT=xi[:, w, dc, off:off + 128],
                                         rhs=wvs["b"][:, dc, :],
                                         start=False, stop=(dc == DC - 1))
                    nc.scalar.copy(
                        vcr[:, tq * HC:(tq + 1) * HC, 0:64],
                        pv[:, 0:C].rearrange("p (h d) -> p h d", h=HC))
                    nc.scalar.copy(
                        vci[:, tq * HC:(tq + 1) * HC, 0:64],
                        pv[:, C:2 * C].rearrange("p (h d) -> p h d", h=HC))

            # ---------------- attention phase ----------------
            # per (head, 512-query window): 8 key-chunk pairs; scores for
            # pair p+1 are emitted before the AV matmuls of pair p so the
            # exp latency is hidden behind ~1.3us of PE work.
            with tc.tile_pool(name="ox", bufs=1) as ox:
                # prefetch o-proj weights now: the scalar queue is idle and
                # SBUF has room once the projection pools wind down.
                ows = {k: ox.tile([128, 2, D], BF16, name=f"ow{k}")
                       for k in ("r", "i", "n")}
                for k in ("r", "i", "n"):
                    nc.scalar.dma_start(ows[k][:], ow[k][:])

                with tc.tile_pool(name="att", bufs=3) as att, \
                     tc.tile_pool(name="attsm", bufs=2) as attsm, \
                     tc.tile_pool(name="ost", bufs=3) as ost, \
                     tc.tile_pool(name="sp", bufs=2, space="PSUM") as sp, \
                     tc.tile_pool(name="avr", bufs=2, space="PSUM") as avrp, \
                     tc.tile_pool(name="avi", bufs=2, space="PSUM") as avip:
                    # one flat software pipeline over every (window, head,
                    # key-chunk-pair): the scores+exp of pair g+1 are always
                    # in flight while the AV matmuls of pair g run, including
                    # across head/window boundaries, so the PE never waits on
                    # the ACT exp.
                    pairs = [(iw, h, p) for iw in range(T5)
                             for h in range(HC) for p in range(NP)]
                    etiles, avst = {}, {}

                    def emit_scores(g):
                        iw, h, p = pairs[g]
                        s = sp.tile([128, 1024], F32, name="s")
                        for j in range(2):
                            nc.tensor.matmul(
                                s[:, ts(j, 512)],
                                lhsT=qkcat[:, HC + h, ts(2 * p + j, 128)],
                                rhs=qkcat[:, h, ts(iw, 512)],
                                start=True, stop=True)
                        es = att.tile([128, 1024], BF16, name="es")
                        nc.scalar.activation(es[:], s[:], EXP, scale=0.125)
                        etiles[g] = es

                    def emit_accum(g):
                        iw, h, p = pairs[g]
                        if p == 0:
                            avst[(iw, h)] = (
                                avrp.tile([128, 512], F32, name="avr"),
                                avip.tile([128, 512], F32, name="avi"))
                        avr, avi = avst[(iw, h)]
                        es = etiles.pop(g)
                        for j in range(2):
                            nc.tensor.matmul(
                                avr[0:65, :],
                                lhsT=vcr[:, (2 * p + j) * HC + h, :],
                                rhs=es[:, ts(j, 512)],
                                start=(p == 0 and j == 0),
                                stop=(p == NP - 1 and j == 1))
                        for j in range(2):
                            nc.tensor.matmul(
                                avi[0:65, :],
                                lhsT=vci[:, (2 * p + j) * HC + h, :],
                                rhs=es[:, ts(j, 512)],
                                start=(p == 0 and j == 0),
                                stop=(p == NP - 1 and j == 1))
                        if p == NP - 1:
                            ucc, up0 = h // 2, (h % 2) * 64
                            dnr = attsm.tile([1, 512], F32, name="dnr")
                            nc.scalar.copy(dnr[:], avr[64:65, :])
                            rec = attsm.tile([1, 512], F32, name="rec")
                            nc.vector.reciprocal_approx_fast(rec[:], dnr[:])
                            bc = attsm.tile([128, 512], F32, name="bc")
                            nc.gpsimd.partition_broadcast(bc[:], rec[:])
                            nc.vector.tensor_mul(
                                urts[iw][up0:up0 + 64, ucc, :],
                                avr[0:64, :], bc[0:64, :])
                            nc.vector.tensor_mul(
                                uits[iw][up0:up0 + 64, ucc, :],
                                avi[0:64, :], bc[64:128, :])

                    emit_scores(0)
                    for g in range(len(pairs)):
                        if g + 1 < len(pairs):
                            emit_scores(g + 1)
                        emit_accum(g)

                    # ---------------- output projection ----------------
                    # PSUM comes from the attention avr/avi pools (same
                    # name, same shape, same banks) so no new PSUM pool has
                    # to wait for the attention pools to drain; 512-column
                    # granularity also splits the final copies/DMAs so the
                    # kernel tail is short.
                    for tq in range(TQ):
                        tslq = ts(tq, 128)
                        w, off = tq // 4, (tq % 4) * 128
                        ur, ui = urts[w], uits[w]
                        usl = slice(off, off + 128)
                        for oc in range(2):
                            osl = ts(oc, 512)
                            por = avrp.tile([128, 512], F32, name="avr")
                            poi = avip.tile([128, 512], F32, name="avi")
                            nc.tensor.matmul(por[:], lhsT=ur[:, 0, usl],
                                             rhs=ows["r"][:, 0, osl],
                                             start=True, stop=False)
                            nc.tensor.matmul(por[:], lhsT=ur[:, 1, usl],
                                             rhs=ows["r"][:, 1, osl],
                                             start=False, stop=False)
                            nc.tensor.matmul(por[:], lhsT=ui[:, 0, usl],
                                             rhs=ows["n"][:, 0, osl],
                                             start=False, stop=False)
                            nc.tensor.matmul(por[:], lhsT=ui[:, 1, usl],
                                             rhs=ows["n"][:, 1, osl],
                                             start=False, stop=True)
                            nc.tensor.matmul(poi[:], lhsT=ur[:, 0, usl],
                                             rhs=ows["i"][:, 0, osl],
                                             start=True, stop=False)
                            nc.tensor.matmul(poi[:], lhsT=ur[:, 1, usl],
                                             rhs=ows["i"][:, 1, osl],
                                             start=False, stop=False)
                            nc.tensor.matmul(poi[:], lhsT=ui[:, 0, usl],
                                             rhs=ows["r"][:, 0, osl],
                                             start=False, stop=False)
                            nc.tensor.matmul(poi[:], lhsT=ui[:, 1, usl],
                                             rhs=ows["r"][:, 1, osl],
                                             start=False, stop=True)
                            st = ost.tile([128, 512], F32, name="st")
                            nc.scalar.copy(st[:], por[:])
                            nc.sync.dma_start(outr_d[tslq, osl], st[:])
                            sti = ost.tile([128, 512], F32, name="sti")
                            nc.vector.tensor_copy(sti[:], poi[:])
                            nc.scalar.dma_start(outi_d[tslq, osl], sti[:])

    nc.compile()
    return nc


def _to_bf16_kxm(arr, parts=128):
    """[K, M] fp32 -> [128, K//128, M] bf16 with K split as (chunk, part)."""
    k, m = arr.shape
    out = arr.reshape(k // parts, parts, m).transpose(1, 0, 2)
    return np.ascontiguousarray(out.astype(ml_dtypes.bfloat16))


def _to_x_slabs(arr):
    """[T, D] fp32 -> [128, T5, DC, 512] bf16: D split as (chunk, part),
    T split into 512-col slabs, slab-major so each slab is contiguous."""
    out = _to_bf16_kxm(arr.T.astype(np.float32))        # [128, DC, T]
    out = out.reshape(128, DC, T5, 512).transpose(0, 2, 1, 3)
    return np.ascontiguousarray(out)


def _rope_tables():
    inv_freq = 1.0 / (10000.0 ** (np.arange(0, HD, 2, dtype=np.float64) / HD))
    invf64 = np.concatenate([inv_freq, inv_freq])          # [64]
    ang = invf64[:, None] * np.arange(T, dtype=np.float64)[None, :]  # [64, T]
    cos2 = np.tile(np.cos(ang), (2, 1)).astype(np.float32)
    sin2 = np.tile(np.sin(ang), (2, 1)).astype(np.float32)
    return np.ascontiguousarray(cos2), np.ascontiguousarray(sin2)


def kernel(x_real, x_imag, q_wr, q_wi, k_wr, k_wi, v_wr, v_wi, o_wr, o_wi):
    global _COMPILED, LAST_RESULTS
    if _COMPILED is None:
        _COMPILED = _build()
    nc = _COMPILED

    cos2, sin2 = _rope_tables()
    xt = {}
    for b in range(B):
        xt[("r", b)] = _to_x_slabs(np.asarray(x_real[b]))
        xt[("i", b)] = _to_x_slabs(np.asarray(x_imag[b]))

    in_maps = []
    for core in range(NCORE):
        b, g = core // TP, core % TP
        cols = slice(g * C, (g + 1) * C)
        m = {"xrT": xt[("r", b)], "xiT": xt[("i", b)],
             "cos2": cos2, "sin2": sin2}
        for nm, wr_, wi_ in (("wq", q_wr, q_wi), ("wk", k_wr, k_wi)):
            m[f"{nm}_r"] = _to_bf16_kxm(np.asarray(wr_[:, cols]))
            m[f"{nm}_i"] = _to_bf16_kxm(np.asarray(wi_[:, cols]))
            m[f"{nm}_n"] = _to_bf16_kxm(-np.asarray(wi_[:, cols]))
        vr_, vi_ = np.asarray(v_wr[:, cols]), np.asarray(v_wi[:, cols])
        m["wv_a"] = _to_bf16_kxm(np.concatenate([vr_, vi_], axis=1))
        m["wv_b"] = _to_bf16_kxm(np.concatenate([-vi_, vr_], axis=1))
        m["ow_r"] = _to_bf16_kxm(np.asarray(o_wr[cols, :]))
        m["ow_i"] = _to_bf16_kxm(np.asarray(o_wi[cols, :]))
        m["ow_n"] = _to_bf16_kxm(-np.asarray(o_wi[cols, :]))
        in_maps.append(m)

    res = run_bass_kernel_spmd(nc, in_maps, core_ids=list(range(NCORE)))
    LAST_RESULTS = res

    final_r = np.zeros((B, T, D), np.float32)
    final_i = np.zeros((B, T, D), np.float32)
    for core in range(NCORE):
        b = core // TP
        final_r[b] += res.results[core]["out_r"]
        final_i[b] += res.results[core]["out_i"]
    return final_r, final_i


# revision 27
# speedup vs baseline: 1.0591x; 1.0285x over previous
"""ComplexAttentionV3 Trainium2 kernel (v3).

Sharding: 8 cores = data-parallel over batch (2) x tensor-parallel over
heads (16 -> 4 per core). Each core computes q/k/v for its 4 heads
(column-sharded projections), local attention, and a row-sharded
o-projection producing a partial [T, D] output; the host sums the 4
partials per batch.

v3 notes vs v2 (559us baseline):
- attention software-pipelined: scores for pair p+1 are emitted before
  the AV matmuls of pair p, so the PE never idles waiting for the ACT
  exp (the 822ns/2jc stall + the resulting p-state down-clock were the
  dominant cost in v2's 272us attention phase);
- softmax denominator fused into the AV matmul as a 65th lhsT column of
  ones (v_real | 1), removing the separate dn matmuls' PSUM pool and
  freeing banks for double-buffered accumulators (av pools bufs=2), so
  the per-(h,iw) normalize chain (DVE recip + gpsimd broadcast + muls)
  runs off the PE critical path;
- attention works on 512-query windows: scores pair tile [128,1024]
  (2 banks, bufs=2) + avr [65,512] + avi [64,512] (1 bank, bufs=2 each)
  = 8 PSUM banks exactly;
- o-proj weights DMA'd at attention start (v2 left them to the o-proj
  phase and stalled 10us); outputs alternate between the sync and
  scalar HWDGE queues (v2 pushed all 16MB through one queue);
- x DMA'd in 512-column slabs interleaved r/i to match first-use order.
"""

import numpy as np
import ml_dtypes

import concourse.bacc as bacc
import concourse.tile as tile
from concourse import mybir
from concourse.bass import ts
from concourse.bass_utils import run_bass_kernel_spmd

B, T, D, H = 2, 2048, 1024, 16
HD = 64
NCORE = 8
TP = 4               # head-parallel degree (per batch)
HC = H // TP         # heads per core = 4
C = HC * HD          # local channels = 256
DC = D // 128        # contraction chunks = 8
TQ = T // 128        # 128-row t-chunks = 16
T5 = T // 512        # 512-col t-chunks = 4
TW = T // 1024       # 1024-col t-chunks = 2
NP = TQ // 2         # key-chunk pairs = 8

F32 = mybir.dt.float32
BF16 = mybir.dt.bfloat16
EXP = mybir.ActivationFunctionType.Exp

LAST_RESULTS = None
_COMPILED = None


def _build():
    nc = bacc.Bacc("TRN2", target_bir_lowering=False, debug=False,
                   num_devices=NCORE)

    def din(name, shape, dt=BF16):
        return nc.dram_tensor(name, shape, dt, kind="ExternalInput").ap()

    xr_d = din("xrT", [128, T5, DC, 512])
    xi_d = din("xiT", [128, T5, DC, 512])
    wq = {k: din(f"wq_{k}", [128, DC, C]) for k in ("r", "i", "n")}
    wk = {k: din(f"wk_{k}", [128, DC, C]) for k in ("r", "i", "n")}
    wv = {k: din(f"wv_{k}", [128, DC, 2 * C]) for k in ("a", "b")}
    ow = {k: din(f"ow_{k}", [128, 2, D]) for k in ("r", "i", "n")}
    cos_d = din("cos2", [128, T], F32)
    sin_d = din("sin2", [128, T], F32)
    outr_d = nc.dram_tensor("out_r", [T, D], F32, kind="ExternalOutput").ap()
    outi_d = nc.dram_tensor("out_i", [T, D], F32, kind="ExternalOutput").ap()

    with tile.TileContext(nc) as tc:
        with tc.tile_pool(name="persist", bufs=1) as persist:
            qkcat = persist.tile([128, 2 * HC, T], BF16, name="qkcat")
            # v_real and v_imag each padded to 65 columns per (key-chunk,
            # head): column 64 is ones. For v_real it computes the softmax
            # denominator into avr partition 64; for v_imag it only pads M
            # to 65 so the matmul stays in the PE's 128-column tile mode
            # (M=64 selects the 64-column mode and every mode switch costs
            # ~95ns). Flat index is tq * HC + h.
            vcr = persist.tile([128, TQ * HC, 65], BF16, name="vcr")
            vci = persist.tile([128, TQ * HC, 65], BF16, name="vci")
            # per-512-query-window u tiles (separate tiles so the o-proj's
            # reads don't pick up a false whole-tile dependency on the last
            # attention window)
            urts = [persist.tile([128, 2, 512], BF16, name=f"urt{w}")
                    for w in range(T5)]
            uits = [persist.tile([128, 2, 512], BF16, name=f"uit{w}")
                    for w in range(T5)]
            nc.vector.memset(vcr[:, :, 64:65], 1.0)
            nc.vector.memset(vci[:, :, 64:65], 1.0)

            # ---------------- projection phase ----------------
            with tc.tile_pool(name="xw", bufs=1) as xw, \
                 tc.tile_pool(name="rt", bufs=1) as rt, \
                 tc.tile_pool(name="pp", bufs=2, space="PSUM") as pp:
                # scalar HWDGE queue: weights + rope tables in first-use
                # order; sync HWDGE queue: x in 512-col slabs, r/i
                # interleaved (q-proj consumes slab s of xr then xi).
                wqs = {k: xw.tile([128, DC, C], BF16, name=f"wq{k}")
                       for k in ("r", "i", "n")}
                wks = {k: xw.tile([128, DC, C], BF16, name=f"wk{k}")
                       for k in ("r", "i", "n")}
                wvs = {k: xw.tile([128, DC, 2 * C], BF16, name=f"wv{k}")
                       for k in ("a", "b")}
                cos = xw.tile([128, T], F32, name="cos")
                sin = xw.tile([128, T], F32, name="sin")
                xr = xw.tile([128, T5, DC, 512], BF16, name="xr")
                xi = xw.tile([128, T5, DC, 512], BF16, name="xi")
                # both HWDGE queues are scheduled by consumption deadline:
                # q-proj consumes a 1MB x slab every ~3.5us from ~12us on
                # (xr then xi per 512-col slab), RoPE needs cos/sin at
                # ~22us, k-proj weights at ~68us, v at ~120us.
                nc.sync.dma_start(wqs["r"][:, 0:4], wq["r"][:, 0:4])
                nc.sync.dma_start(wqs["r"][:, 4:8], wq["r"][:, 4:8])
                nc.sync.dma_start(xr[:, 0, 0:4], xr_d[:, 0, 0:4])
                nc.sync.dma_start(xr[:, 0, 4:8], xr_d[:, 0, 4:8])
                for sl in range(1, T5):
                    nc.sync.dma_start(xr[:, sl], xr_d[:, sl])
                nc.sync.dma_start(xi[:, 2], xi_d[:, 2])
                nc.sync.dma_start(xi[:, 3], xi_d[:, 3])
                for k in ("i", "n"):
                    nc.scalar.dma_start(wqs[k][:], wq[k][:])
                nc.scalar.dma_start(xi[:, 0, 0:4], xi_d[:, 0, 0:4])
                nc.scalar.dma_start(xi[:, 0, 4:8], xi_d[:, 0, 4:8])
                nc.scalar.dma_start(xi[:, 1], xi_d[:, 1])
                nc.scalar.dma_start(cos[:], cos_d[:])
                nc.scalar.dma_start(sin[:], sin_d[:])
                for k in ("r", "i", "n"):
                    nc.scalar.dma_start(wks[k][:], wk[k][:])
                for k in ("a", "b"):
                    nc.scalar.dma_start(wvs[k][:], wv[k][:])

                # q/k projections (transposed [c, t]) + RoPE into qkcat
                for wsrc, hbase in ((wqs, 0), (wks, HC)):
                    for cc in range(2):
                        h0, h1 = hbase + 2 * cc, hbase + 2 * cc + 1
                        for tw in range(TW):
                            pqr = pp.tile([128, 1024], F32, name="ppa")
                            pqi = pp.tile([128, 1024], F32, name="ppb")
                            for half in range(2):
                                sl = 2 * tw + half
                                psl = ts(half, 512)
                                # both xr-consuming chains before the
                                # xi-consuming ones: 16 matmuls of work
                                # while the xi slab DMA is still in flight
                                for dc in range(DC):
                                    nc.tensor.matmul(
                                        pqr[:, psl],
                                        lhsT=wsrc["r"][:, dc, ts(cc, 128)],
                                        rhs=xr[:, sl, dc, :],
                                        start=(dc == 0), stop=False)
                                for dc in range(DC):
                                    nc.tensor.matmul(
                                        pqi[:, psl],
                                        lhsT=wsrc["i"][:, dc, ts(cc, 128)],
                                        rhs=xr[:, sl, dc, :],
                                        start=(dc == 0), stop=False)
                                for dc in range(DC):
                                    nc.tensor.matmul(
                                        pqr[:, psl],
                                        lhsT=wsrc["n"][:, dc, ts(cc, 128)],
                                        rhs=xi[:, sl, dc, :],
                                        start=False, stop=(dc == DC - 1))
                                for dc in range(DC):
                                    nc.tensor.matmul(
                                        pqi[:, psl],
                                        lhsT=wsrc["r"][:, dc, ts(cc, 128)],
                                        rhs=xi[:, sl, dc, :],
                                        start=False, stop=(dc == DC - 1))
                            tsl = ts(tw, 1024)
                            t1 = rt.tile([128, 1024], F32, name="t1")
                            t2 = rt.tile([128, 1024], F32, name="t2")
                            t3 = rt.tile([128, 1024], F32, name="t3")
                            t4 = rt.tile([128, 1024], F32, name="t4")
                            nc.vector.tensor_mul(t1[:], pqr[:], cos[:, tsl])
                            nc.vector.tensor_mul(t2[:], pqi[:], sin[:, tsl])
                            nc.vector.tensor_mul(t3[:], pqr[:], sin[:, tsl])
                            nc.vector.tensor_mul(t4[:], pqi[:], cos[:, tsl])
                            nc.vector.tensor_sub(qkcat[0:64, h0, tsl],
                                                 t1[0:64, :], t2[0:64, :])
                            nc.vector.tensor_sub(qkcat[0:64, h1, tsl],
                                                 t1[64:128, :], t2[64:128, :])
                            nc.vector.tensor_add(qkcat[64:128, h0, tsl],
                                                 t3[0:64, :], t4[0:64, :])
                            nc.vector.tensor_add(qkcat[64:128, h1, tsl],
                                                 t3[64:128, :], t4[64:128, :])

                # v projection: natural [t, c], rhs packed [wvr | wvi]
                for tq in range(TQ):
                    pv = pp.tile([128, 1024], F32, name="ppa")
                    pvs = pv[:, 0:512]
                    w, off = tq // 4, (tq % 4) * 128
                    for dc in range(DC):
                        nc.tensor.matmul(pvs,
                                         lhsT=xr[:, w, dc, off:off + 128],
                                         rhs=wvs["a"][:, dc, :],
                                         start=(dc == 0), stop=False)
                    for dc in range(DC):
                        nc.tensor.matmul(pvs,
                                         lhsT=xi[:, w, dc, off:off + 128],
                                         rhs=wvs["b"][:, dc, :],
                                         start=False, stop=(dc == DC - 1))
                    nc.scalar.copy(
                        vcr[:, tq * HC:(tq + 1) * HC, 0:64],
                        pv[:, 0:C].rearrange("p (h d) -> p h d", h=HC))
                    nc.scalar.copy(
                        vci[:, tq * HC:(tq + 1) * HC, 0:64],
                        pv[:, C:2 * C].rearrange("p (h d) -> p h d", h=HC))

            # ---------------- attention phase ----------------
            # per (head, 512-query window): 8 key-chunk pairs; scores for
            # pair p+1 are emitted before the AV matmuls of pair p so the
            # exp latency is hidden behind ~1.3us of PE work.
            with tc.tile_pool(name="ox", bufs=1) as ox:
                # prefetch o-proj weights now: the scalar queue is idle and
                # SBUF has room once the projection pools wind down.
                ows = {k: ox.tile([128, 2, D], BF16, name=f"ow{k}")
                       for k in ("r", "i", "n")}
                for k in ("r", "i", "n"):
                    nc.scalar.dma_start(ows[k][:], ow[k][:])

                with tc.tile_pool(name="att", bufs=3) as att, \
                     tc.tile_pool(name="attsm", bufs=2) as attsm, \
                     tc.tile_pool(name="ost", bufs=3) as ost, \
                     tc.tile_pool(name="sp", bufs=2, space="PSUM") as sp, \
                     tc.tile_pool(name="avr", bufs=2, space="PSUM") as avrp, \
                     tc.tile_pool(name="avi", bufs=2, space="PSUM") as avip:
                    # one flat software pipeline over every (window, head,
                    # key-chunk-pair): the scores+exp of pair g+1 are always
                    # in flight while the AV matmuls of pair g run, including
                    # across head/window boundaries, so the PE never waits on
                    # the ACT exp.
                    pairs = [(iw, h, p) for iw in range(T5)
                             for h in range(HC) for p in range(NP)]
                    etiles, avst = {}, {}

                    def emit_scores(g):
                        iw, h, p = pairs[g]
                        s = sp.tile([128, 1024], F32, name="s")
                        for j in range(2):
                            nc.tensor.matmul(
                                s[:, ts(j, 512)],
                                lhsT=qkcat[:, HC + h, ts(2 * p + j, 128)],
                                rhs=qkcat[:, h, ts(iw, 512)],
                                start=True, stop=True)
                        es = att.tile([128, 1024], BF16, name="es")
                        nc.scalar.activation(es[:], s[:], EXP, scale=0.125)
                        etiles[g] = es

                    def emit_accum(g):
                        iw, h, p = pairs[g]
                        if p == 0:
                            avst[(iw, h)] = (
                                avrp.tile([128, 512], F32, name="avr"),
                                avip.tile([128, 512], F32, name="avi"))
                        avr, avi = avst[(iw, h)]
                        es = etiles.pop(g)
                        for j in range(2):
                            nc.tensor.matmul(
                                avr[0:65, :],
                                lhsT=vcr[:, (2 * p + j) * HC + h, :],
                                rhs=es[:, ts(j, 512)],
                                start=(p == 0 and j == 0),
                                stop=(p == NP - 1 and j == 1))
                        for j in range(2):
                            nc.tensor.matmul(
                                avi[0:65, :],
                                lhsT=vci[:, (2 * p + j) * HC + h, :],
                                rhs=es[:, ts(j, 512)],
                                start=(p == 0 and j == 0),
                                stop=(p == NP - 1 and j == 1))
                        if p == NP - 1:
                            ucc, up0 = h // 2, (h % 2) * 64
                            # dn-row copy on DVE, not ACT: the ACT engine
                            # runs at ~86% on exps and a copy there stalls
                            # the pipeline at every window boundary
                            dnr = attsm.tile([1, 512], F32, name="dnr")
                            nc.vector.tensor_copy(dnr[:], avr[64:65, :])
                            rec = attsm.tile([1, 512], F32, name="rec")
                            nc.vector.reciprocal_approx_fast(rec[:], dnr[:])
                            bc = attsm.tile([128, 512], F32, name="bc")
                            nc.gpsimd.partition_broadcast(bc[:], rec[:])
                            nc.vector.tensor_mul(
                                urts[iw][up0:up0 + 64, ucc, :],
                                avr[0:64, :], bc[0:64, :])
                            nc.vector.tensor_mul(
                                uits[iw][up0:up0 + 64, ucc, :],
                                avi[0:64, :], bc[64:128, :])

                    emit_scores(0)
                    for g in range(len(pairs)):
                        if g + 1 < len(pairs):
                            emit_scores(g + 1)
                        emit_accum(g)

                    # ---------------- output projection ----------------
                    # PSUM comes from the attention avr/avi pools (same
                    # name, same shape, same banks) so no new PSUM pool has
                    # to wait for the attention pools to drain; 512-column
                    # granularity also splits the final copies/DMAs so the
                    # kernel tail is short.
                    for tq in range(TQ):
                        tslq = ts(tq, 128)
                        w, off = tq // 4, (tq % 4) * 128
                        ur, ui = urts[w], uits[w]
                        usl = slice(off, off + 128)
                        for oc in range(2):
                            osl = ts(oc, 512)
                            por = avrp.tile([128, 512], F32, name="avr")
                            poi = avip.tile([128, 512], F32, name="avi")
                            nc.tensor.matmul(por[:], lhsT=ur[:, 0, usl],
                                             rhs=ows["r"][:, 0, osl],
                                             start=True, stop=False)
                            nc.tensor.matmul(por[:], lhsT=ur[:, 1, usl],
                                             rhs=ows["r"][:, 1, osl],
                                             start=False, stop=False)
                            nc.tensor.matmul(por[:], lhsT=ui[:, 0, usl],
                                             rhs=ows["n"][:, 0, osl],
                                             start=False, stop=False)
                            nc.tensor.matmul(por[:], lhsT=ui[:, 1, usl],
                                             rhs=ows["n"][:, 1, osl],
                                             start=False, stop=True)
                            nc.tensor.matmul(poi[:], lhsT=ur[:, 0, usl],
                                             rhs=ows["i"][:, 0, osl],
                                             start=True, stop=False)
                            nc.tensor.matmul(poi[:], lhsT=ur[:, 1, usl],
                                             rhs=ows["i"][:, 1, osl],
                                             start=False, stop=False)
                            nc.tensor.matmul(poi[:], lhsT=ui[:, 0, usl],
                                             rhs=ows["r"][:, 0, osl],
                                             start=False, stop=False)
                            nc.tensor.matmul(poi[:], lhsT=ui[:, 1, usl],
                                             rhs=ows["r"][:, 1, osl],
                                             start=False, stop=True)
                            st = ost.tile([128, 512], F32, name="st")
                            nc.scalar.copy(st[:], por[:])
                            nc.sync.dma_start(outr_d[tslq, osl], st[:])
                            sti = ost.tile([128, 512], F32, name="sti")
                            nc.vector.tensor_copy(sti[:], poi[:])
                            nc.scalar.dma_start(outi_d[tslq, osl], sti[:])

    nc.compile()
    return nc


def _to_bf16_kxm(arr, parts=128):
    """[K, M] fp32 -> [128, K//128, M] bf16 with K split as (chunk, part)."""
    k, m = arr.shape
    out = arr.reshape(k // parts, parts, m).transpose(1, 0, 2)
    return np.ascontiguousarray(out.astype(ml_dtypes.bfloat16))


def _to_x_slabs(arr):
    """[T, D] fp32 -> [128, T5, DC, 512] bf16: D split as (chunk, part),
    T split into 512-col slabs, slab-major so each slab is contiguous."""
    out = _to_bf16_kxm(arr.T.astype(np.float32))        # [128, DC, T]
    out = out.reshape(128, DC, T5, 512).transpose(0, 2, 1, 3)
    return np.ascontiguousarray(out)


def _rope_tables():
    inv_freq = 1.0 / (10000.0 ** (np.arange(0, HD, 2, dtype=np.float64) / HD))
    invf64 = np.concatenate([inv_freq, inv_freq])          # [64]
    ang = invf64[:, None] * np.arange(T, dtype=np.float64)[None, :]  # [64, T]
    cos2 = np.tile(np.cos(ang), (2, 1)).astype(np.float32)
    sin2 = np.tile(np.sin(ang), (2, 1)).astype(np.float32)
    return np.ascontiguousarray(cos2), np.ascontiguousarray(sin2)


def kernel(x_real, x_imag, q_wr, q_wi, k_wr, k_wi, v_wr, v_wi, o_wr, o_wi):
    global _COMPILED, LAST_RESULTS
    if _COMPILED is None:
        _COMPILED = _build()
    nc = _COMPILED

    cos2, sin2 = _rope_tables()
    xt = {}
    for b in range(B):
        xt[("r", b)] = _to_x_slabs(np.asarray(x_real[b]))
        xt[("i", b)] = _to_x_slabs(np.asarray(x_imag[b]))

    in_maps = []
    for core in range(NCORE):
        b, g = core // TP, core % TP
        cols = slice(g * C, (g + 1) * C)
        m = {"xrT": xt[("r", b)], "xiT": xt[("i", b)],
             "cos2": cos2, "sin2": sin2}
        for nm, wr_, wi_ in (("wq", q_wr, q_wi), ("wk", k_wr, k_wi)):
            m[f"{nm}_r"] = _to_bf16_kxm(np.asarray(wr_[:, cols]))
            m[f"{nm}_i"] = _to_bf16_kxm(np.asarray(wi_[:, cols]))
            m[f"{nm}_n"] = _to_bf16_kxm(-np.asarray(wi_[:, cols]))
        vr_, vi_ = np.asarray(v_wr[:, cols]), np.asarray(v_wi[:, cols])
        m["wv_a"] = _to_bf16_kxm(np.concatenate([vr_, vi_], axis=1))
        m["wv_b"] = _to_bf16_kxm(np.concatenate([-vi_, vr_], axis=1))
        m["ow_r"] = _to_bf16_kxm(np.asarray(o_wr[cols, :]))
        m["ow_i"] = _to_bf16_kxm(np.asarray(o_wi[cols, :]))
        m["ow_n"] = _to_bf16_kxm(-np.asarray(o_wi[cols, :]))
        in_maps.append(m)

    res = run_bass_kernel_spmd(nc, in_maps, core_ids=list(range(NCORE)))
    LAST_RESULTS = res

    final_r = np.zeros((B, T, D), np.float32)
    final_i = np.zeros((B, T, D), np.float32)
    for core in range(NCORE):
        b = core // TP
        final_r[b] += res.results[core]["out_r"]
        final_i[b] += res.results[core]["out_i"]
    return final_r, final_i


# revision 31
# speedup vs baseline: 1.0667x; 1.0071x over previous
"""ComplexAttentionV3 Trainium2 kernel (v3).

Sharding: 8 cores = data-parallel over batch (2) x tensor-parallel over
heads (16 -> 4 per core). Each core computes q/k/v for its 4 heads
(column-sharded projections), local attention, and a row-sharded
o-projection producing a partial [T, D] output; the host sums the 4
partials per batch.

v3 notes vs v2 (559us baseline):
- attention software-pipelined: scores for pair p+1 are emitted before
  the AV matmuls of pair p, so the PE never idles waiting for the ACT
  exp (the 822ns/2jc stall + the resulting p-state down-clock were the
  dominant cost in v2's 272us attention phase);
- softmax denominator fused into the AV matmul as a 65th lhsT column of
  ones (v_real | 1), removing the separate dn matmuls' PSUM pool and
  freeing banks for double-buffered accumulators (av pools bufs=2), so
  the per-(h,iw) normalize chain (DVE recip + gpsimd broadcast + muls)
  runs off the PE critical path;
- attention works on 512-query windows: scores pair tile [128,1024]
  (2 banks, bufs=2) + avr [65,512] + avi [64,512] (1 bank, bufs=2 each)
  = 8 PSUM banks exactly;
- o-proj weights DMA'd at attention start (v2 left them to the o-proj
  phase and stalled 10us); outputs alternate between the sync and
  scalar HWDGE queues (v2 pushed all 16MB through one queue);
- x DMA'd in 512-column slabs interleaved r/i to match first-use order.
"""

import numpy as np
import ml_dtypes

import concourse.bacc as bacc
import concourse.tile as tile
from concourse import mybir
from concourse.bass import ts
from concourse.bass_utils import run_bass_kernel_spmd

B, T, D, H = 2, 2048, 1024, 16
HD = 64
NCORE = 8
TP = 4               # head-parallel degree (per batch)
HC = H // TP         # heads per core = 4
C = HC * HD          # local channels = 256
DC = D // 128        # contraction chunks = 8
TQ = T // 128        # 128-row t-chunks = 16
T5 = T // 512        # 512-col t-chunks = 4
TW = T // 1024       # 1024-col t-chunks = 2
NP = TQ // 2         # key-chunk pairs = 8

F32 = mybir.dt.float32
BF16 = mybir.dt.bfloat16
EXP = mybir.ActivationFunctionType.Exp

LAST_RESULTS = None
_COMPILED = None


def _build():
    nc = bacc.Bacc("TRN2", target_bir_lowering=False, debug=False,
                   num_devices=NCORE)

    def din(name, shape, dt=BF16):
        return nc.dram_tensor(name, shape, dt, kind="ExternalInput").ap()

    xr_d = din("xrT", [128, T5, DC, 512])
    xi_d = din("xiT", [128, T5, DC, 512])
    wq = {k: din(f"wq_{k}", [128, DC, C]) for k in ("r", "i", "n")}
    wk = {k: din(f"wk_{k}", [128, DC, C]) for k in ("r", "i", "n")}
    wv = {k: din(f"wv_{k}", [128, DC, 2 * C]) for k in ("a", "b")}
    ow = {k: din(f"ow_{k}", [128, 2, D]) for k in ("r", "i", "n")}
    cos_d = din("cos2", [128, T])
    sin_d = din("sin2", [128, T])
    outr_d = nc.dram_tensor("out_r", [T, D], F32, kind="ExternalOutput").ap()
    outi_d = nc.dram_tensor("out_i", [T, D], F32, kind="ExternalOutput").ap()

    with tile.TileContext(nc) as tc:
        with tc.tile_pool(name="persist", bufs=1) as persist:
            qkcat = persist.tile([128, 2 * HC, T], BF16, name="qkcat")
            # v_real and v_imag each padded to 65 columns per (key-chunk,
            # head): column 64 is ones. For v_real it computes the softmax
            # denominator into avr partition 64; for v_imag it only pads M
            # to 65 so the matmul stays in the PE's 128-column tile mode
            # (M=64 selects the 64-column mode and every mode switch costs
            # ~95ns). Flat index is tq * HC + h.
            vcr = persist.tile([128, TQ * HC, 65], BF16, name="vcr")
            vci = persist.tile([128, TQ * HC, 65], BF16, name="vci")
            # per-512-query-window u tiles (separate tiles so the o-proj's
            # reads don't pick up a false whole-tile dependency on the last
            # attention window)
            urts = [persist.tile([128, 2, 512], BF16, name=f"urt{w}")
                    for w in range(T5)]
            uits = [persist.tile([128, 2, 512], BF16, name=f"uit{w}")
                    for w in range(T5)]
            nc.vector.memset(vcr[:, :, 64:65], 1.0)
            nc.vector.memset(vci[:, :, 64:65], 1.0)

            # ---------------- projection phase ----------------
            with tc.tile_pool(name="xw", bufs=1) as xw, \
                 tc.tile_pool(name="rt", bufs=1) as rt, \
                 tc.tile_pool(name="pp", bufs=2, space="PSUM") as pp:
                # scalar HWDGE queue: weights + rope tables in first-use
                # order; sync HWDGE queue: x in 512-col slabs, r/i
                # interleaved (q-proj consumes slab s of xr then xi).
                wqs = {k: xw.tile([128, DC, C], BF16, name=f"wq{k}")
                       for k in ("r", "i", "n")}
                wks = {k: xw.tile([128, DC, C], BF16, name=f"wk{k}")
                       for k in ("r", "i", "n")}
                wvs = {k: xw.tile([128, DC, 2 * C], BF16, name=f"wv{k}")
                       for k in ("a", "b")}
                cos = xw.tile([128, T], BF16, name="cos")
                sin = xw.tile([128, T], BF16, name="sin")
                xr = xw.tile([128, T5, DC, 512], BF16, name="xr")
                xi = xw.tile([128, T5, DC, 512], BF16, name="xi")
                # both HWDGE queues are scheduled by consumption deadline:
                # q-proj consumes a 1MB x slab every ~3.5us from ~12us on
                # (xr then xi per 512-col slab), RoPE needs cos/sin at
                # ~22us, k-proj weights at ~68us, v at ~120us.
                nc.sync.dma_start(wqs["r"][:, 0:4], wq["r"][:, 0:4])
                nc.sync.dma_start(wqs["r"][:, 4:8], wq["r"][:, 4:8])
                nc.sync.dma_start(xr[:, 0, 0:4], xr_d[:, 0, 0:4])
                nc.sync.dma_start(xr[:, 0, 4:8], xr_d[:, 0, 4:8])
                for sl in range(1, T5):
                    nc.sync.dma_start(xr[:, sl], xr_d[:, sl])
                for k in ("i", "n"):
                    nc.scalar.dma_start(wqs[k][:], wq[k][:])
                nc.scalar.dma_start(xi[:, 0, 0:4], xi_d[:, 0, 0:4])
                nc.scalar.dma_start(xi[:, 0, 4:8], xi_d[:, 0, 4:8])
                nc.scalar.dma_start(xi[:, 1], xi_d[:, 1])
                nc.scalar.dma_start(cos[:], cos_d[:])
                nc.scalar.dma_start(sin[:], sin_d[:])
                nc.scalar.dma_start(xi[:, 2], xi_d[:, 2])
                nc.scalar.dma_start(xi[:, 3], xi_d[:, 3])
                for k in ("r", "i", "n"):
                    nc.scalar.dma_start(wks[k][:], wk[k][:])
                for k in ("a", "b"):
                    nc.scalar.dma_start(wvs[k][:], wv[k][:])

                # q/k projections (transposed [c, t]) + RoPE into qkcat
                for wsrc, hbase in ((wqs, 0), (wks, HC)):
                    for cc in range(2):
                        h0, h1 = hbase + 2 * cc, hbase + 2 * cc + 1
                        for tw in range(TW):
                            pqr = pp.tile([128, 1024], F32, name="ppa")
                            pqi = pp.tile([128, 1024], F32, name="ppb")
                            for half in range(2):
                                sl = 2 * tw + half
                                psl = ts(half, 512)
                                # both xr-consuming chains before the
                                # xi-consuming ones: 16 matmuls of work
                                # while the xi slab DMA is still in flight
                                for dc in range(DC):
                                    nc.tensor.matmul(
                                        pqr[:, psl],
                                        lhsT=wsrc["r"][:, dc, ts(cc, 128)],
                                        rhs=xr[:, sl, dc, :],
                                        start=(dc == 0), stop=False)
                                for dc in range(DC):
                                    nc.tensor.matmul(
                                        pqi[:, psl],
                                        lhsT=wsrc["i"][:, dc, ts(cc, 128)],
                                        rhs=xr[:, sl, dc, :],
                                        start=(dc == 0), stop=False)
                                for dc in range(DC):
                                    nc.tensor.matmul(
                                        pqr[:, psl],
                                        lhsT=wsrc["n"][:, dc, ts(cc, 128)],
                                        rhs=xi[:, sl, dc, :],
                                        start=False, stop=(dc == DC - 1))
                                for dc in range(DC):
                                    nc.tensor.matmul(
                                        pqi[:, psl],
                                        lhsT=wsrc["r"][:, dc, ts(cc, 128)],
                                        rhs=xi[:, sl, dc, :],
                                        start=False, stop=(dc == DC - 1))
                            tsl = ts(tw, 1024)
                            t1 = rt.tile([128, 1024], F32, name="t1")
                            t2 = rt.tile([128, 1024], F32, name="t2")
                            t3 = rt.tile([128, 1024], F32, name="t3")
                            t4 = rt.tile([128, 1024], F32, name="t4")
                            nc.vector.tensor_mul(t1[:], pqr[:], cos[:, tsl])
                            nc.vector.tensor_mul(t2[:], pqi[:], sin[:, tsl])
                            nc.vector.tensor_mul(t3[:], pqr[:], sin[:, tsl])
                            nc.vector.tensor_mul(t4[:], pqi[:], cos[:, tsl])
                            nc.vector.tensor_sub(qkcat[0:64, h0, tsl],
                                                 t1[0:64, :], t2[0:64, :])
                            nc.vector.tensor_sub(qkcat[0:64, h1, tsl],
                                                 t1[64:128, :], t2[64:128, :])
                            nc.vector.tensor_add(qkcat[64:128, h0, tsl],
                                                 t3[0:64, :], t4[0:64, :])
                            nc.vector.tensor_add(qkcat[64:128, h1, tsl],
                                                 t3[64:128, :], t4[64:128, :])

                # v projection: natural [t, c], rhs packed [wvr | wvi]
                for tq in range(TQ):
                    pv = pp.tile([128, 1024], F32, name="ppa")
                    pvs = pv[:, 0:512]
                    w, off = tq // 4, (tq % 4) * 128
                    for dc in range(DC):
                        nc.tensor.matmul(pvs,
                                         lhsT=xr[:, w, dc, off:off + 128],
                                         rhs=wvs["a"][:, dc, :],
                                         start=(dc == 0), stop=False)
                    for dc in range(DC):
                        nc.tensor.matmul(pvs,
                                         lhsT=xi[:, w, dc, off:off + 128],
                                         rhs=wvs["b"][:, dc, :],
                                         start=False, stop=(dc == DC - 1))
                    nc.scalar.copy(
                        vcr[:, tq * HC:(tq + 1) * HC, 0:64],
                        pv[:, 0:C].rearrange("p (h d) -> p h d", h=HC))
                    nc.scalar.copy(
                        vci[:, tq * HC:(tq + 1) * HC, 0:64],
                        pv[:, C:2 * C].rearrange("p (h d) -> p h d", h=HC))

            # ---------------- attention phase ----------------
            # per (head, 512-query window): 8 key-chunk pairs; scores for
            # pair p+1 are emitted before the AV matmuls of pair p so the
            # exp latency is hidden behind ~1.3us of PE work.
            with tc.tile_pool(name="ox", bufs=1) as ox:
                # prefetch o-proj weights now: the scalar queue is idle and
                # SBUF has room once the projection pools wind down.
                ows = {k: ox.tile([128, 2, D], BF16, name=f"ow{k}")
                       for k in ("r", "i", "n")}
                for k in ("r", "i", "n"):
                    nc.scalar.dma_start(ows[k][:], ow[k][:])

                with tc.tile_pool(name="att", bufs=3) as att, \
                     tc.tile_pool(name="attsm", bufs=2) as attsm, \
                     tc.tile_pool(name="ost", bufs=3) as ost, \
                     tc.tile_pool(name="sp", bufs=2, space="PSUM") as sp, \
                     tc.tile_pool(name="avr", bufs=2, space="PSUM") as avrp, \
                     tc.tile_pool(name="avi", bufs=2, space="PSUM") as avip:
                    # one flat software pipeline over every (window, head,
                    # key-chunk-pair): the scores+exp of pair g+1 are always
                    # in flight while the AV matmuls of pair g run, including
                    # across head/window boundaries, so the PE never waits on
                    # the ACT exp.
                    pairs = [(iw, h, p) for iw in range(T5)
                             for h in range(HC) for p in range(NP)]
                    etiles, avst = {}, {}

                    def emit_scores(g):
                        iw, h, p = pairs[g]
                        s = sp.tile([128, 1024], F32, name="s")
                        for j in range(2):
                            nc.tensor.matmul(
                                s[:, ts(j, 512)],
                                lhsT=qkcat[:, HC + h, ts(2 * p + j, 128)],
                                rhs=qkcat[:, h, ts(iw, 512)],
                                start=True, stop=True)
                        es = att.tile([128, 1024], BF16, name="es")
                        nc.scalar.activation(es[:], s[:], EXP, scale=0.125)
                        etiles[g] = es

                    def emit_accum(g):
                        iw, h, p = pairs[g]
                        if p == 0:
                            avst[(iw, h)] = (
                                avrp.tile([128, 512], F32, name="avr"),
                                avip.tile([128, 512], F32, name="avi"))
                        avr, avi = avst[(iw, h)]
                        es = etiles.pop(g)
                        for j in range(2):
                            nc.tensor.matmul(
                                avr[0:65, :],
                                lhsT=vcr[:, (2 * p + j) * HC + h, :],
                                rhs=es[:, ts(j, 512)],
                                start=(p == 0 and j == 0),
                                stop=(p == NP - 1 and j == 1))
                        for j in range(2):
                            nc.tensor.matmul(
                                avi[0:65, :],
                                lhsT=vci[:, (2 * p + j) * HC + h, :],
                                rhs=es[:, ts(j, 512)],
                                start=(p == 0 and j == 0),
                                stop=(p == NP - 1 and j == 1))
                        if p == NP - 1:
                            ucc, up0 = h // 2, (h % 2) * 64
                            # dn-row copy on DVE, not ACT: the ACT engine
                            # runs at ~86% on exps and a copy there stalls
                            # the pipeline at every window boundary
                            dnr = attsm.tile([1, 512], F32, name="dnr")
                            nc.vector.tensor_copy(dnr[:], avr[64:65, :])
                            rec = attsm.tile([1, 512], F32, name="rec")
                            nc.vector.reciprocal_approx_fast(rec[:], dnr[:])
                            bc = attsm.tile([128, 512], F32, name="bc")
                            nc.gpsimd.partition_broadcast(bc[:], rec[:])
                            nc.vector.tensor_mul(
                                urts[iw][up0:up0 + 64, ucc, :],
                                avr[0:64, :], bc[0:64, :])
                            nc.vector.tensor_mul(
                                uits[iw][up0:up0 + 64, ucc, :],
                                avi[0:64, :], bc[64:128, :])

                    emit_scores(0)
                    for g in range(len(pairs)):
                        if g + 1 < len(pairs):
                            emit_scores(g + 1)
                        emit_accum(g)

                    # ---------------- output projection ----------------
                    # PSUM comes from the attention avr/avi pools (same
                    # name, same shape, same banks) so no new PSUM pool has
                    # to wait for the attention pools to drain; 512-column
                    # granularity also splits the final copies/DMAs so the
                    # kernel tail is short.
                    for tq in range(TQ):
                        tslq = ts(tq, 128)
                        w, off = tq // 4, (tq % 4) * 128
                        ur, ui = urts[w], uits[w]
                        usl = slice(off, off + 128)
                        for oc in range(2):
                            osl = ts(oc, 512)
                            por = avrp.tile([128, 512], F32, name="avr")
                            poi = avip.tile([128, 512], F32, name="avi")
                            nc.tensor.matmul(por[:], lhsT=ur[:, 0, usl],
                                             rhs=ows["r"][:, 0, osl],
                                             start=True, stop=False)
                            nc.tensor.matmul(por[:], lhsT=ur[:, 1, usl],
                                             rhs=ows["r"][:, 1, osl],
                                             start=False, stop=False)
                            nc.tensor.matmul(por[:], lhsT=ui[:, 0, usl],
                                             rhs=ows["n"][:, 0, osl],
                                             start=False, stop=False)
                            nc.tensor.matmul(por[:], lhsT=ui[:, 1, usl],
                                             rhs=ows["n"][:, 1, osl],
                                             start=False, stop=True)
                            nc.tensor.matmul(poi[:], lhsT=ur[:, 0, usl],
                                             rhs=ows["i"][:, 0, osl],
                                             start=True, stop=False)
                            nc.tensor.matmul(poi[:], lhsT=ur[:, 1, usl],
                                             rhs=ows["i"][:, 1, osl],
                                             start=False, stop=False)
                            nc.tensor.matmul(poi[:], lhsT=ui[:, 0, usl],
                                             rhs=ows["r"][:, 0, osl],
                                             start=False, stop=False)
                            nc.tensor.matmul(poi[:], lhsT=ui[:, 1, usl],
                                             rhs=ows["r"][:, 1, osl],
                                             start=False, stop=True)
                            st = ost.tile([128, 512], F32, name="st")
                            nc.scalar.copy(st[:], por[:])
                            nc.sync.dma_start(outr_d[tslq, osl], st[:])
                            sti = ost.tile([128, 512], F32, name="sti")
                            nc.vector.tensor_copy(sti[:], poi[:])
                            nc.scalar.dma_start(outi_d[tslq, osl], sti[:])

    nc.compile()
    return nc


def _to_bf16_kxm(arr, parts=128):
    """[K, M] fp32 -> [128, K//128, M] bf16 with K split as (chunk, part)."""
    k, m = arr.shape
    out = arr.reshape(k // parts, parts, m).transpose(1, 0, 2)
    return np.ascontiguousarray(out.astype(ml_dtypes.bfloat16))


def _to_x_slabs(arr):
    """[T, D] fp32 -> [128, T5, DC, 512] bf16: D split as (chunk, part),
    T split into 512-col slabs, slab-major so each slab is contiguous."""
    out = _to_bf16_kxm(arr.T.astype(np.float32))        # [128, DC, T]
    out = out.reshape(128, DC, T5, 512).transpose(0, 2, 1, 3)
    return np.ascontiguousarray(out)


def _rope_tables():
    inv_freq = 1.0 / (10000.0 ** (np.arange(0, HD, 2, dtype=np.float64) / HD))
    invf64 = np.concatenate([inv_freq, inv_freq])          # [64]
    ang = invf64[:, None] * np.arange(T, dtype=np.float64)[None, :]  # [64, T]
    cos2 = np.tile(np.cos(ang), (2, 1)).astype(ml_dtypes.bfloat16)
    sin2 = np.tile(np.sin(ang), (2, 1)).astype(ml_dtypes.bfloat16)
    return np.ascontiguousarray(cos2), np.ascontiguousarray(sin2)


def kernel(x_real, x_imag, q_wr, q_wi, k_wr, k_wi, v_wr, v_wi, o_wr, o_wi):
    global _COMPILED, LAST_RESULTS
    if _COMPILED is None:
        _COMPILED = _build()
    nc = _COMPILED

    cos2, sin2 = _rope_tables()
    xt = {}
    for b in range(B):
        xt[("r", b)] = _to_x_slabs(np.asarray(x_real[b]))
        xt[("i", b)] = _to_x_slabs(np.asarray(x_imag[b]))

    in_maps = []
    for core in range(NCORE):
        b, g = core // TP, core % TP
        cols = slice(g * C, (g + 1) * C)
        m = {"xrT": xt[("r", b)], "xiT": xt[("i", b)],
             "cos2": cos2, "sin2": sin2}
        for nm, wr_, wi_ in (("wq", q_wr, q_wi), ("wk", k_wr, k_wi)):
            m[f"{nm}_r"] = _to_bf16_kxm(np.asarray(wr_[:, cols]))
            m[f"{nm}_i"] = _to_bf16_kxm(np.asarray(wi_[:, cols]))
            m[f"{nm}_n"] = _to_bf16_kxm(-np.asarray(wi_[:, cols]))
        vr_, vi_ = np.asarray(v_wr[:, cols]), np.asarray(v_wi[:, cols])
        m["wv_a"] = _to_bf16_kxm(np.concatenate([vr_, vi_], axis=1))
        m["wv_b"] = _to_bf16_kxm(np.concatenate([-vi_, vr_], axis=1))
        m["ow_r"] = _to_bf16_kxm(np.asarray(o_wr[cols, :]))
        m["ow_i"] = _to_bf16_kxm(np.asarray(o_wi[cols, :]))
        m["ow_n"] = _to_bf16_kxm(-np.asarray(o_wi[cols, :]))
        in_maps.append(m)

    res = run_bass_kernel_spmd(nc, in_maps, core_ids=list(range(NCORE)))
    LAST_RESULTS = res

    final_r = np.zeros((B, T, D), np.float32)
    final_i = np.zeros((B, T, D), np.float32)
    for core in range(NCORE):
        b = core // TP
        final_r[b] += res.results[core]["out_r"]
        final_i[b] += res.results[core]["out_i"]
    return final_r, final_i
